# revision 60
# baseline (speedup 1.0000x reference)
"""Trainium2 Bass kernel for MixActivConv2d (mixed-precision fake-quant + 1x1 conv).

Reference computation:
  sel = x[:, ch]                                   # gather 8 channels
  activ = sum_i softmax(aa)[i] * uq(sel, bit_i)    # global-minmax fake quant
  x_q = x with sel channels replaced by activ
  w_q = sum_i softmax(aw)[i] * uq(w, bit_i)
  out = conv1x1(x_q, w_q)  ==  w_q[256,256] @ x_q[b, 256, 4096]

Strategy (8 cores, data-parallel over batch, 4 batches/core):
  - channels permuted so the 8 selected channels sit in the last 8 rows of
    the second K-half; x streams in fp16, the GEMM runs in fp16 (1 cyc/row
    on the PE vs 4 for fp32), output is written fp16 and upcast on host
  - the fake-quant of the selected channels runs on device in exact fp32
    (per-op IEEE arithmetic, magic-number RNE rounding identical to the
    reference), producing fp16 activ rows that are DMA-scattered into the
    rhs stream before the K1 matmul passes
  - weight fake-quant also on device (fp32 exact -> fp16 lhsT)
  - global min/max of sel and W plus the handful of scalar constants
    (1/scale_i, folded softmax-blend factors) are computed host-side in
    exact IEEE fp32 (bit-identical to the on-device scalar chain they
    replace), so no cross-core reduction or collective is needed
  - DMA cost scales with bytes-per-partition, so transfers are shaped wide
    (128 partitions) and spread over the SP / ACT / Pool-SWDGE queues; ACT-
    and Pool-queue transfers occupy their engines, so the placement trades
    off against quant work and PSUM evictions (DVE/ACT only: GPSIMD cannot
    read PSUM on real TRN2)
"""

import sys
from contextlib import ExitStack

import numpy as np

sys.path.insert(0, "/opt/trn_rl_repo")

import concourse.bass as bass  # noqa: E402
import concourse.mybir as mybir  # noqa: E402
import concourse.tile as tile  # noqa: E402
from concourse import bacc  # noqa: E402

NCORES = 8
B, C, H, W = 32, 256, 64, 64
HW = H * W  # 4096
BPC = B // NCORES  # batches per core = 4
NSEL = 8
QMAX = (3.0, 15.0, 255.0)  # 2^bit - 1 for bits (2, 4, 8)
MAGIC = 12582912.0  # 1.5 * 2**23: x + MAGIC - MAGIC == rne-round(x) for |x| < 2^22
F32 = mybir.dt.float32
F16 = mybir.dt.float16
ALU = mybir.AluOpType
ACTF = mybir.ActivationFunctionType

# cb column layout per path (sel path at col 0, W path at col 8):
#  +0: -mn   +1..3: inv_i = 1/scale_i   +4..6: k_i = sw_i*scale_i   +7: mn


def _emit_quant(nc, tmps, src, cb, sc0, dst, dc0, cbase, ncols, danger, mid, tail):
    """Fake-quant src[:, sc0:sc0+ncols] -> dst[:, dc0:dc0+ncols] (fp16 out).

    Danger ops (pre-round, bit-exact IEEE per op, no fused nonzero
    scale+bias): u = src + (-mn); r_i = u*inv_i; rho_i = r_i + MAGIC.
    Run on DVE/gpsimd as tensor ops or on ACT as single-op activations
    (a neutral second operand keeps each op exactly IEEE).
    Mid ops (post-round; the MAGIC offset must come off BEFORE any scaling
    — exact there, catastrophic after): p_i = (rho_i - MAGIC)*k_i — fused
    tensor_scalar on DVE/gp, or two exact single-op activations on ACT.
    Tail (DVE/gp): s = p0 + p1;  dst = (p2 + mn) + s  (STT writes fp16).
    """
    u, rho = tmps
    ssl = (slice(0, 128), slice(sc0, sc0 + ncols))
    tsl = (slice(0, 128), slice(sc0, sc0 + ncols))
    dsl = (slice(0, 128), slice(dc0, dc0 + ncols))
    first = None
    if danger == "act":
        first = nc.scalar.activation(
            u[tsl], src[ssl], ACTF.Identity, bias=cb[:, cbase : cbase + 1]
        )
        for i in range(3):
            nc.scalar.activation(
                rho[i][tsl], u[tsl], ACTF.Copy,
                scale=cb[:, cbase + 1 + i : cbase + 2 + i],
            )
            nc.scalar.activation(rho[i][tsl], rho[i][tsl], ACTF.Copy, bias=MAGIC)
    else:
        eng = nc.vector if danger == "dve" else nc.gpsimd
        first = eng.tensor_scalar(
            u[tsl], src[ssl], cb[:, cbase : cbase + 1], None, op0=ALU.add
        )
        for i in range(3):
            eng.tensor_scalar(
                rho[i][tsl], u[tsl], cb[:, cbase + 1 + i : cbase + 2 + i], None,
                op0=ALU.mult,
            )
            eng.tensor_scalar(rho[i][tsl], rho[i][tsl], MAGIC, None, op0=ALU.add)
    if mid == "act":
        for i in range(3):
            nc.scalar.activation(rho[i][tsl], rho[i][tsl], ACTF.Copy, bias=-MAGIC)
            nc.scalar.activation(
                rho[i][tsl], rho[i][tsl], ACTF.Copy,
                scale=cb[:, cbase + 4 + i : cbase + 5 + i],
            )
    else:
        eng2 = nc.vector if mid == "dve" else nc.gpsimd
        for i in range(3):
            eng2.tensor_scalar(
                rho[i][tsl], rho[i][tsl], MAGIC, cb[:, cbase + 4 + i : cbase + 5 + i],
                op0=ALU.subtract, op1=ALU.mult,
            )
    if tail == "gp":
        # STT has no Pool opcode: use TT,TT,TS so the chunk never touches DVE
        nc.gpsimd.tensor_add(rho[0][tsl], rho[0][tsl], rho[1][tsl])
        nc.gpsimd.tensor_add(rho[2][tsl], rho[2][tsl], rho[0][tsl])
        last = nc.gpsimd.tensor_scalar(
            dst[dsl], rho[2][tsl], cb[:, cbase + 7 : cbase + 8], None, op0=ALU.add
        )
    else:
        nc.vector.tensor_add(rho[0][tsl], rho[0][tsl], rho[1][tsl])
        last = nc.vector.scalar_tensor_tensor(
            dst[dsl], rho[2][tsl], cb[:, cbase + 7 : cbase + 8], rho[0][tsl],
            op0=ALU.add, op1=ALU.add,
        )
    return first, last


def _kernel_body(ctx, tc, x0_ap, x1_ap, selloc_ap, w_ap, cb_ap, out_ap):
    nc = tc.nc

    const = ctx.enter_context(tc.tile_pool(name="const", bufs=1))
    stage = ctx.enter_context(tc.tile_pool(name="stage", bufs=3))
    psB = ctx.enter_context(tc.tile_pool(name="psB", bufs=4, space="PSUM"))

    # ---- input DMAs ----
    # SP queue: constants first (they gate all quant work), then the x
    # stream for b0/b1/b3 and b2's K1 half, with b1/b3 emitted after the
    # quant so each batch's scatter can sit right behind its x chunks.
    cbrow = const.tile([1, 16], F32)
    nc.sync.dma_start(cbrow[:], cb_ap)
    selloc = const.tile([128, 1024], F32)
    nc.sync.dma_start(selloc[:, 0:512], selloc_ap[:, 0:512])
    wtside = const.tile([128, 2 * C], F32)
    nc.sync.dma_start(wtside[:], w_ap)
    nc.scalar.dma_start(selloc[:, 512:1024], selloc_ap[:, 512:1024])

    rhs0 = const.tile([128, BPC * HW], F16, name="rhs0", tag="rhs0")
    rhs1 = const.tile([128, BPC * HW], F16, name="rhs1", tag="rhs1")

    def load_x(b, eng):
        cs = slice(b * HW, (b + 1) * HW)
        eng.dma_start(rhs0[:, cs], x0_ap[:, cs])
        eng.dma_start(rhs1[:, cs], x1_ap[:, cs])

    def scatter(b):
        # q-outer packed layout: activ rows q*32+b*8+j -> rhs1 row 120+j,
        # cols b*HW + q*1024.  One [8,1024] DMA per (b, q): per-partition
        # bytes stay small, which is what the DMA cost scales with.  Two
        # chunks each on the ACT and Pool queues run in parallel.
        # Must follow b's x1 chunk (WAW on the sel rows).
        for q in range(4):
            eng = (nc.scalar if q < 2 else nc.gpsimd) if b == 0 else nc.gpsimd
            eng.dma_start(
                rhs1[120:128, b * HW + q * 1024 : b * HW + (q + 1) * 1024],
                activ16[q * 32 + b * 8 : q * 32 + (b + 1) * 8, :],
            )

    load_x(0, nc.sync)

    # quant scratch (shared across chunks; chunks touch disjoint columns)
    tmps_s = (
        const.tile([128, 1024], F32, name="qus", tag="qus"),
        [const.tile([128, 1024], F32, name=f"qrs{i}", tag=f"qrs{i}") for i in range(3)],
    )
    tmps_w = (
        const.tile([128, 512], F32, name="quw", tag="quw"),
        [const.tile([128, 512], F32, name=f"qrw{i}", tag=f"qrw{i}") for i in range(3)],
    )

    # lhsT layout (m-major, matching host wt): [K0m0 | K1m0 | K0m1 | K1m1]
    lhsT = const.tile([128, 512], F16, name="lhsT", tag="lhsT")
    activ16 = const.tile([128, 1024], F16, name="activ16", tag="activ16")
    cb = const.tile([128, 16], F32)
    with tc.high_priority():
        nc.gpsimd.partition_broadcast(cb[:], cbrow[0:1, :])

    # ---- sel quant (packed [128,1024]: p = q*32 + b*8 + j, cols = hw%1024)
    # activ gates every K1 pass; W-m0 only gates the first K0 pass, which
    # the PE reaches later — so the Pool engine runs its sel share FIRST.
    _emit_quant(nc, tmps_s, selloc, cb, 0, activ16, 0, 0, 576, "dve", "dve", "dve")
    _emit_quant(nc, tmps_s, selloc, cb, 576, activ16, 576, 0, 176, "act", "act", "dve")
    selp_first, selp_last = _emit_quant(
        nc, tmps_s, selloc, cb, 752, activ16, 752, 0, 272, "gp", "gp", "gp"
    )

    wm0_first, _ = _emit_quant(
        nc, tmps_w, wtside, cb, 0, lhsT, 0, 8, 256, "gp", "gp", "gp"
    )
    from concourse.tile import add_dep_helper

    add_dep_helper(wm0_first.ins, selp_last.ins, sync=False, reason="pool: sel first")

    # W m1 chunk on DVE after its sel share (PE needs lhsT-m1 several us in)
    _emit_quant(nc, tmps_w, wtside, cb, 256, lhsT, 256, 8, 256, "dve", "dve", "dve")

    # x stream: b1, b3 and b2's K1 half on SP; b2's K0 half on the Pool
    # queue in its idle window between quant and evictions
    with tc.high_priority():
        scatter(0)
    load_x(1, nc.sync)
    scatter(1)
    nc.gpsimd.dma_start(rhs0[:, 2 * HW : 3 * HW], x0_ap[:, 2 * HW : 3 * HW])
    nc.sync.dma_start(rhs1[:, 2 * HW : 3 * HW], x1_ap[:, 2 * HW : 3 * HW])
    scatter(2)
    load_x(3, nc.sync)
    scatter(3)

    # ---- main GEMM: per (b, m): 4 psum tiles [128,1024], K0+K1, evict ----
    # NOTE: GPSIMD cannot read PSUM on real TRN2 — evictions are DVE/ACT only
    evict_sched = ["dve", "act"] * 16
    # out halves [128,2048] alternate queues so transfers overlap
    out_qs = {
        0: (nc.scalar, nc.gpsimd), 1: (nc.sync, nc.scalar),
        2: (nc.sync, nc.gpsimd), 3: (nc.sync, nc.scalar),
    }
    ei = 0
    for b in (0, 1, 2, 3):
        for m in range(2):
            outsb = stage.tile([128, HW], F16, name="outsb", tag="outsb")
            is_last = b == 3 and m == 1
            if is_last:
                # drain each tile right after its eviction (shorter tail)
                for t in range(4):
                    pt = psB.tile([128, 1024], F32, name="ptile", tag="ptile")
                    for h in range(2):
                        c0 = b * HW + t * 1024 + h * 512
                        nc.tensor.matmul(
                            pt[:, h * 512 : (h + 1) * 512],
                            lhsT[:, m * 256 : m * 256 + 128],
                            rhs0[:, c0 : c0 + 512], start=True, stop=False,
                        )
                    for h in range(2):
                        c0 = b * HW + t * 1024 + h * 512
                        nc.tensor.matmul(
                            pt[:, h * 512 : (h + 1) * 512],
                            lhsT[:, m * 256 + 128 : m * 256 + 256],
                            rhs1[:, c0 : c0 + 512], start=False, stop=True,
                        )
                    osl = outsb[:, t * 1024 : (t + 1) * 1024]
                    evl = ("dve", "act", "dve", "act")[t]
                    if evl == "act":
                        nc.scalar.copy(osl, pt[:])
                    elif evl == "dve":
                        nc.vector.tensor_copy(osl, pt[:])
                    else:
                        nc.gpsimd.tensor_copy(osl, pt[:])
                    eng = (nc.sync, nc.scalar, nc.sync, nc.scalar)[t]
                    eng.dma_start(
                        out_ap[b, m * 128 : (m + 1) * 128, t * 1024 : (t + 1) * 1024],
                        osl,
                    )
                continue
            for t in range(4):
                pt = psB.tile([128, 1024], F32, name="ptile", tag="ptile")
                for h in range(2):
                    c0 = b * HW + t * 1024 + h * 512
                    nc.tensor.matmul(
                        pt[:, h * 512 : (h + 1) * 512],
                        lhsT[:, m * 256 : m * 256 + 128],
                        rhs0[:, c0 : c0 + 512],
                        start=True, stop=False,
                    )
                for h in range(2):
                    c0 = b * HW + t * 1024 + h * 512
                    nc.tensor.matmul(
                        pt[:, h * 512 : (h + 1) * 512],
                        lhsT[:, m * 256 + 128 : m * 256 + 256],
                        rhs1[:, c0 : c0 + 512],
                        start=False, stop=True,
                    )
                ev = evict_sched[ei]
                ei += 1
                osl = outsb[:, t * 1024 : (t + 1) * 1024]
                if ev == "act":
                    nc.scalar.copy(osl, pt[:])
                elif ev == "dve":
                    nc.vector.tensor_copy(osl, pt[:])
                else:
                    nc.gpsimd.tensor_copy(osl, pt[:])
            for hh in range(2):
                out_qs[b][hh].dma_start(
                    out_ap[b, m * 128 : (m + 1) * 128, hh * 2048 : (hh + 1) * 2048],
                    outsb[:, hh * 2048 : (hh + 1) * 2048],
                )


def build_program(ch=None, reps=1):
    nc = bacc.Bacc(
        "TRN2", target_bir_lowering=False, debug=False, num_devices=NCORES
    )
    x0_t = nc.dram_tensor("x0", [128, BPC * HW], F16, kind="ExternalInput").ap()
    x1_t = nc.dram_tensor("x1", [128, BPC * HW], F16, kind="ExternalInput").ap()
    selloc_t = nc.dram_tensor("selloc", [128, 1024], F32, kind="ExternalInput").ap()
    w_t = nc.dram_tensor("wt", [128, 2 * C], F32, kind="ExternalInput").ap()
    cb_t = nc.dram_tensor("cb", [1, 16], F32, kind="ExternalInput").ap()
    out_t = nc.dram_tensor("out", [BPC, C, HW], F16, kind="ExternalOutput").ap()
    with tile.TileContext(nc) as tc:
        with ExitStack() as ctx:
            _kernel_body(ctx, tc, x0_t, x1_t, selloc_t, w_t, cb_t, out_t)
    nc.compile()
    return nc


def _f32(v):
    return np.float32(v)


def _host_consts(vals, mn, mx, sw):
    """Exact-IEEE fp32 constants for one quant path -> 8 floats."""
    rng = _f32(mx) - _f32(mn)
    scale = [rng / _f32(q) for q in QMAX]
    inv = [_f32(1.0) / s for s in scale]
    k = [_f32(sw[i]) * scale[i] for i in range(3)]
    vals[0] = -_f32(mn)
    vals[1:4] = inv
    vals[4:7] = k
    vals[7] = _f32(mn)


def _softmax32(a):
    a = np.asarray(a, dtype=np.float32)
    e = np.exp(a - a.max(), dtype=np.float32)
    return (e / e.sum(dtype=np.float32)).astype(np.float32)


def make_in_maps(x, alpha_activ, alpha_weight, conv_weight, selected_channels):
    x = np.ascontiguousarray(np.asarray(x, dtype=np.float32).reshape(B, C, HW))
    ch = [int(v) for v in np.asarray(selected_channels).ravel()]
    chset = set(ch)
    nonsel = [c for c in range(C) if c not in chset]
    P = np.array(nonsel + ch, dtype=np.int64)  # sel channels at rows 248:256

    sel = x[:, ch, :]  # [32, 8, 4096] fp32 exact
    smn, smx = sel.min(), sel.max()
    wmat = np.asarray(conv_weight, dtype=np.float32).reshape(C, C)
    wmn, wmx = wmat.min(), wmat.max()

    cbrow = np.zeros((1, 16), dtype=np.float32)
    _host_consts(cbrow[0, 0:8], smn, smx, _softmax32(alpha_activ))
    _host_consts(cbrow[0, 8:16], wmn, wmx, _softmax32(alpha_weight))

    # W^T with permuted input channels, m-major chunks: [K0m0|K1m0|K0m1|K1m1]
    wperm = np.ascontiguousarray(wmat[:, P].T)  # [256(k), 256(m)]
    wt = np.ascontiguousarray(
        np.hstack([
            wperm[0:128, 0:128], wperm[128:256, 0:128],
            wperm[0:128, 128:256], wperm[128:256, 128:256],
        ])
    )

    xp = x[:, P, :].astype(np.float16)  # [32, 256, 4096] fp16, permuted

    in_maps = []
    for c in range(NCORES):
        xs = xp[c * BPC : (c + 1) * BPC]  # [4, 256, 4096]
        x0 = np.ascontiguousarray(xs[:, 0:128, :].transpose(1, 0, 2).reshape(128, -1))
        x1 = np.ascontiguousarray(xs[:, 128:256, :].transpose(1, 0, 2).reshape(128, -1))
        # selloc layout (q-outer): partition p = q*32 + b*8 + j holds
        # sel[core*4+b, j, q*1024 : (q+1)*1024]
        sl = sel[c * BPC : (c + 1) * BPC].reshape(BPC, NSEL, 4, 1024)
        selloc = np.ascontiguousarray(sl.transpose(2, 0, 1, 3).reshape(128, 1024))
        in_maps.append({"x0": x0, "x1": x1, "selloc": selloc, "wt": wt, "cb": cbrow})
    return ch, in_maps


def kernel(x, alpha_activ, alpha_weight, conv_weight, selected_channels):
    from concourse.bass_utils import run_bass_kernel_spmd

    ch, in_maps = make_in_maps(
        x, alpha_activ, alpha_weight, conv_weight, selected_channels
    )
    nc = build_program(ch)
    res = run_bass_kernel_spmd(nc, in_maps, core_ids=list(range(NCORES)))
    outs = [
        res.results[c]["out"].astype(np.float32).reshape(BPC, C, H, W)
        for c in range(NCORES)
    ]
    return np.concatenate(outs, axis=0)


# revision 73
# speedup vs baseline: 1.0166x; 1.0166x over previous
"""Trainium2 Bass kernel for MixActivConv2d (mixed-precision fake-quant + 1x1 conv).

Reference computation:
  sel = x[:, ch]                                   # gather 8 channels
  activ = sum_i softmax(aa)[i] * uq(sel, bit_i)    # global-minmax fake quant
  x_q = x with sel channels replaced by activ
  w_q = sum_i softmax(aw)[i] * uq(w, bit_i)
  out = conv1x1(x_q, w_q)  ==  w_q[256,256] @ x_q[b, 256, 4096]

Strategy (8 cores, data-parallel over batch, 4 batches/core):
  - channels permuted so the 8 selected channels sit in the last 8 rows of
    the second K-half; x streams in fp16, the GEMM runs in fp16 (1 cyc/row
    on the PE vs 4 for fp32), output is written fp16 and upcast on host
  - the fake-quant of the selected channels runs on device in exact fp32
    (per-op IEEE arithmetic, magic-number RNE rounding identical to the
    reference), producing fp16 activ rows that are DMA-scattered into the
    rhs stream before the K1 matmul passes
  - weight fake-quant also on device (fp32 exact -> fp16 lhsT)
  - global min/max of sel and W plus the handful of scalar constants
    (1/scale_i, folded softmax-blend factors) are computed host-side in
    exact IEEE fp32 (bit-identical to the on-device scalar chain they
    replace), so no cross-core reduction or collective is needed
  - DMA cost scales with bytes-per-partition, so transfers are shaped wide
    (128 partitions) and spread over the SP / ACT / Pool-SWDGE queues; ACT-
    and Pool-queue transfers occupy their engines, so the placement trades
    off against quant work and PSUM evictions (DVE/ACT only: GPSIMD cannot
    read PSUM on real TRN2)
"""

import sys
from contextlib import ExitStack

import numpy as np

sys.path.insert(0, "/opt/trn_rl_repo")

import concourse.bass as bass  # noqa: E402
import concourse.mybir as mybir  # noqa: E402
import concourse.tile as tile  # noqa: E402
from concourse import bacc  # noqa: E402

NCORES = 8
B, C, H, W = 32, 256, 64, 64
HW = H * W  # 4096
BPC = B // NCORES  # batches per core = 4
NSEL = 8
QMAX = (3.0, 15.0, 255.0)  # 2^bit - 1 for bits (2, 4, 8)
MAGIC = 12582912.0  # 1.5 * 2**23: x + MAGIC - MAGIC == rne-round(x) for |x| < 2^22
F32 = mybir.dt.float32
F16 = mybir.dt.float16
ALU = mybir.AluOpType
ACTF = mybir.ActivationFunctionType

# cb column layout per path (sel path at col 0, W path at col 8):
#  +0: -mn   +1..3: inv_i = 1/scale_i   +4..6: k_i = sw_i*scale_i   +7: mn


def _emit_quant(nc, tmps, src, cb, sc0, dst, dc0, cbase, ncols, danger, mid, tail):
    """Fake-quant src[:, sc0:sc0+ncols] -> dst[:, dc0:dc0+ncols] (fp16 out).

    Danger ops (pre-round, bit-exact IEEE per op, no fused nonzero
    scale+bias): u = src + (-mn); r_i = u*inv_i; rho_i = r_i + MAGIC.
    Run on DVE/gpsimd as tensor ops or on ACT as single-op activations
    (a neutral second operand keeps each op exactly IEEE).
    Mid ops (post-round; the MAGIC offset must come off BEFORE any scaling
    — exact there, catastrophic after): p_i = (rho_i - MAGIC)*k_i — fused
    tensor_scalar on DVE/gp, or two exact single-op activations on ACT.
    Tail (DVE/gp): s = p0 + p1;  dst = (p2 + mn) + s  (STT writes fp16).
    """
    u, rho = tmps
    ssl = (slice(0, 128), slice(sc0, sc0 + ncols))
    tsl = (slice(0, 128), slice(sc0, sc0 + ncols))
    dsl = (slice(0, 128), slice(dc0, dc0 + ncols))
    first = None
    if danger == "act":
        first = nc.scalar.activation(
            u[tsl], src[ssl], ACTF.Identity, bias=cb[:, cbase : cbase + 1]
        )
        for i in range(3):
            nc.scalar.activation(
                rho[i][tsl], u[tsl], ACTF.Copy,
                scale=cb[:, cbase + 1 + i : cbase + 2 + i],
            )
            nc.scalar.activation(rho[i][tsl], rho[i][tsl], ACTF.Copy, bias=MAGIC)
    else:
        eng = nc.vector if danger == "dve" else nc.gpsimd
        first = eng.tensor_scalar(
            u[tsl], src[ssl], cb[:, cbase : cbase + 1], None, op0=ALU.add
        )
        for i in range(3):
            eng.tensor_scalar(
                rho[i][tsl], u[tsl], cb[:, cbase + 1 + i : cbase + 2 + i], None,
                op0=ALU.mult,
            )
            eng.tensor_scalar(rho[i][tsl], rho[i][tsl], MAGIC, None, op0=ALU.add)
    if mid == "act":
        for i in range(3):
            nc.scalar.activation(rho[i][tsl], rho[i][tsl], ACTF.Copy, bias=-MAGIC)
            nc.scalar.activation(
                rho[i][tsl], rho[i][tsl], ACTF.Copy,
                scale=cb[:, cbase + 4 + i : cbase + 5 + i],
            )
    else:
        eng2 = nc.vector if mid == "dve" else nc.gpsimd
        for i in range(3):
            eng2.tensor_scalar(
                rho[i][tsl], rho[i][tsl], MAGIC, cb[:, cbase + 4 + i : cbase + 5 + i],
                op0=ALU.subtract, op1=ALU.mult,
            )
    if tail == "gp":
        # STT has no Pool opcode: use TT,TT,TS so the chunk never touches DVE
        nc.gpsimd.tensor_add(rho[0][tsl], rho[0][tsl], rho[1][tsl])
        nc.gpsimd.tensor_add(rho[2][tsl], rho[2][tsl], rho[0][tsl])
        last = nc.gpsimd.tensor_scalar(
            dst[dsl], rho[2][tsl], cb[:, cbase + 7 : cbase + 8], None, op0=ALU.add
        )
    else:
        nc.vector.tensor_add(rho[0][tsl], rho[0][tsl], rho[1][tsl])
        last = nc.vector.scalar_tensor_tensor(
            dst[dsl], rho[2][tsl], cb[:, cbase + 7 : cbase + 8], rho[0][tsl],
            op0=ALU.add, op1=ALU.add,
        )
    return first, last


def _kernel_body(ctx, tc, x0_ap, x1_ap, selloc_ap, w_ap, cb_ap, out_ap):
    nc = tc.nc

    const = ctx.enter_context(tc.tile_pool(name="const", bufs=1))
    stage = ctx.enter_context(tc.tile_pool(name="stage", bufs=4))
    psB = ctx.enter_context(tc.tile_pool(name="psB", bufs=4, space="PSUM"))

    # ---- input DMAs ----
    # SP queue: constants first (they gate all quant work), then the x
    # stream for b0/b1/b3 and b2's K1 half, with b1/b3 emitted after the
    # quant so each batch's scatter can sit right behind its x chunks.
    cbrow = const.tile([1, 16], F32)
    nc.sync.dma_start(cbrow[:], cb_ap)
    selloc = const.tile([128, 1024], F32)
    nc.sync.dma_start(selloc[:, 0:512], selloc_ap[:, 0:512])
    wtside = const.tile([128, 2 * C], F32)
    nc.sync.dma_start(wtside[:], w_ap)
    nc.scalar.dma_start(selloc[:, 512:1024], selloc_ap[:, 512:1024])

    rhs0 = const.tile([128, BPC * HW], F16, name="rhs0", tag="rhs0")
    rhs1 = const.tile([128, BPC * HW], F16, name="rhs1", tag="rhs1")

    def load_x(b, eng):
        cs = slice(b * HW, (b + 1) * HW)
        eng.dma_start(rhs0[:, cs], x0_ap[:, cs])
        eng.dma_start(rhs1[:, cs], x1_ap[:, cs])

    def scatter(b):
        # q-outer packed layout: activ rows q*32+b*8+j -> rhs1 row 120+j,
        # cols b*HW + q*1024.  One [8,1024] DMA per (b, q): per-partition
        # bytes stay small, which is what the DMA cost scales with.  Two
        # chunks each on the ACT and Pool queues run in parallel.
        # Must follow b's x1 chunk (WAW on the sel rows).
        for q in range(4):
            eng = (nc.scalar if q < 2 else nc.gpsimd) if b == 0 else nc.gpsimd
            eng.dma_start(
                rhs1[120:128, b * HW + q * 1024 : b * HW + (q + 1) * 1024],
                activ16[q * 32 + b * 8 : q * 32 + (b + 1) * 8, :],
            )

    load_x(0, nc.sync)

    # quant scratch (shared across chunks; chunks touch disjoint columns)
    tmps_s = (
        const.tile([128, 1024], F32, name="qus", tag="qus"),
        [const.tile([128, 1024], F32, name=f"qrs{i}", tag=f"qrs{i}") for i in range(3)],
    )
    tmps_w = (
        const.tile([128, 512], F32, name="quw", tag="quw"),
        [const.tile([128, 512], F32, name=f"qrw{i}", tag=f"qrw{i}") for i in range(3)],
    )

    # lhsT layout (m-major, matching host wt): [K0m0 | K1m0 | K0m1 | K1m1]
    lhsT = const.tile([128, 512], F16, name="lhsT", tag="lhsT")
    activ16 = const.tile([128, 1024], F16, name="activ16", tag="activ16")
    cb = const.tile([128, 16], F32)
    with tc.high_priority():
        nc.gpsimd.partition_broadcast(cb[:], cbrow[0:1, :])

    # ---- sel quant (packed [128,1024]: p = q*32 + b*8 + j, cols = hw%1024)
    # activ gates every K1 pass; W-m0 only gates the first K0 pass, which
    # the PE reaches later — so the Pool engine runs its sel share FIRST.
    _emit_quant(nc, tmps_s, selloc, cb, 0, activ16, 0, 0, 576, "dve", "dve", "dve")
    _emit_quant(nc, tmps_s, selloc, cb, 576, activ16, 576, 0, 176, "act", "act", "dve")
    selp_first, selp_last = _emit_quant(
        nc, tmps_s, selloc, cb, 752, activ16, 752, 0, 272, "gp", "gp", "gp"
    )

    wm0_first, _ = _emit_quant(
        nc, tmps_w, wtside, cb, 0, lhsT, 0, 8, 128, "gp", "gp", "gp"
    )
    _emit_quant(nc, tmps_w, wtside, cb, 128, lhsT, 128, 8, 128, "gp", "gp", "gp")
    from concourse.tile import add_dep_helper

    add_dep_helper(wm0_first.ins, selp_last.ins, sync=False, reason="pool: sel first")

    # W m1 chunk on DVE after its sel share (PE needs lhsT-m1 several us in)
    _emit_quant(nc, tmps_w, wtside, cb, 256, lhsT, 256, 8, 256, "dve", "dve", "dve")

    # x stream: b1, b3 and b2's K1 half on SP; b2's K0 half on the Pool
    # queue in its idle window between quant and evictions
    nc.gpsimd.dma_start(rhs0[:, 2 * HW : 3 * HW], x0_ap[:, 2 * HW : 3 * HW])
    with tc.high_priority():
        scatter(0)
    load_x(1, nc.sync)
    scatter(1)
    nc.sync.dma_start(rhs1[:, 2 * HW : 3 * HW], x1_ap[:, 2 * HW : 3 * HW])
    scatter(2)
    load_x(3, nc.sync)
    scatter(3)

    # ---- main GEMM: per (b, m): 4 psum tiles [128,1024], K0+K1, evict ----
    # NOTE: GPSIMD cannot read PSUM on real TRN2 — evictions are DVE/ACT only
    evict_sched = ["dve", "act"] * 16
    # out halves [128,2048] alternate queues so transfers overlap
    out_qs = {
        0: (nc.scalar, nc.gpsimd), 1: (nc.sync, nc.gpsimd),
        2: (nc.sync, nc.gpsimd), 3: (nc.sync, nc.scalar),
    }
    ei = 0
    for b in (0, 1, 2, 3):
        for m in range(2):
            outsb = stage.tile([128, HW], F16, name="outsb", tag="outsb")
            is_last = b == 3 and m == 1
            if is_last:
                # drain each tile right after its eviction; the final tile
                # evicts per half on DVE+ACT in parallel with quarter drains
                # so the post-PE tail is as short as possible
                for t in range(4):
                    pt = psB.tile([128, 1024], F32, name="ptile", tag="ptile")
                    for h in range(2):
                        c0 = b * HW + t * 1024 + h * 512
                        nc.tensor.matmul(
                            pt[:, h * 512 : (h + 1) * 512],
                            lhsT[:, m * 256 : m * 256 + 128],
                            rhs0[:, c0 : c0 + 512], start=True, stop=False,
                        )
                    for h in range(2):
                        c0 = b * HW + t * 1024 + h * 512
                        nc.tensor.matmul(
                            pt[:, h * 512 : (h + 1) * 512],
                            lhsT[:, m * 256 + 128 : m * 256 + 256],
                            rhs1[:, c0 : c0 + 512], start=False, stop=True,
                        )
                    if t == 3:
                        for h in range(2):
                            c1 = t * 1024 + h * 512
                            osl = outsb[:, c1 : c1 + 512]
                            if h == 0:
                                nc.scalar.copy(osl, pt[:, 0:512])
                            else:
                                nc.vector.tensor_copy(osl, pt[:, 512:1024])
                            eng = (nc.sync, nc.scalar)[h]
                            eng.dma_start(
                                out_ap[b, m * 128 : (m + 1) * 128, c1 : c1 + 512],
                                osl,
                            )
                        continue
                    osl = outsb[:, t * 1024 : (t + 1) * 1024]
                    evl = ("dve", "act", "dve")[t]
                    if evl == "act":
                        nc.scalar.copy(osl, pt[:])
                    else:
                        nc.vector.tensor_copy(osl, pt[:])
                    eng = (nc.sync, nc.scalar, nc.sync)[t]
                    eng.dma_start(
                        out_ap[b, m * 128 : (m + 1) * 128, t * 1024 : (t + 1) * 1024],
                        osl,
                    )
                continue
            for t in range(4):
                pt = psB.tile([128, 1024], F32, name="ptile", tag="ptile")
                for h in range(2):
                    c0 = b * HW + t * 1024 + h * 512
                    nc.tensor.matmul(
                        pt[:, h * 512 : (h + 1) * 512],
                        lhsT[:, m * 256 : m * 256 + 128],
                        rhs0[:, c0 : c0 + 512],
                        start=True, stop=False,
                    )
                for h in range(2):
                    c0 = b * HW + t * 1024 + h * 512
                    nc.tensor.matmul(
                        pt[:, h * 512 : (h + 1) * 512],
                        lhsT[:, m * 256 + 128 : m * 256 + 256],
                        rhs1[:, c0 : c0 + 512],
                        start=False, stop=True,
                    )
                ev = evict_sched[ei]
                ei += 1
                osl = outsb[:, t * 1024 : (t + 1) * 1024]
                if ev == "act":
                    nc.scalar.copy(osl, pt[:])
                elif ev == "dve":
                    nc.vector.tensor_copy(osl, pt[:])
                else:
                    nc.gpsimd.tensor_copy(osl, pt[:])
            for hh in range(2):
                out_qs[b][hh].dma_start(
                    out_ap[b, m * 128 : (m + 1) * 128, hh * 2048 : (hh + 1) * 2048],
                    outsb[:, hh * 2048 : (hh + 1) * 2048],
                )


def build_program(ch=None, reps=1):
    nc = bacc.Bacc(
        "TRN2", target_bir_lowering=False, debug=False, num_devices=NCORES
    )
    x0_t = nc.dram_tensor("x0", [128, BPC * HW], F16, kind="ExternalInput").ap()
    x1_t = nc.dram_tensor("x1", [128, BPC * HW], F16, kind="ExternalInput").ap()
    selloc_t = nc.dram_tensor("selloc", [128, 1024], F32, kind="ExternalInput").ap()
    w_t = nc.dram_tensor("wt", [128, 2 * C], F32, kind="ExternalInput").ap()
    cb_t = nc.dram_tensor("cb", [1, 16], F32, kind="ExternalInput").ap()
    out_t = nc.dram_tensor("out", [BPC, C, HW], F16, kind="ExternalOutput").ap()
    with tile.TileContext(nc) as tc:
        with ExitStack() as ctx:
            _kernel_body(ctx, tc, x0_t, x1_t, selloc_t, w_t, cb_t, out_t)
    nc.compile()
    return nc


def _f32(v):
    return np.float32(v)


def _host_consts(vals, mn, mx, sw):
    """Exact-IEEE fp32 constants for one quant path -> 8 floats."""
    rng = _f32(mx) - _f32(mn)
    scale = [rng / _f32(q) for q in QMAX]
    inv = [_f32(1.0) / s for s in scale]
    k = [_f32(sw[i]) * scale[i] for i in range(3)]
    vals[0] = -_f32(mn)
    vals[1:4] = inv
    vals[4:7] = k
    vals[7] = _f32(mn)


def _softmax32(a):
    a = np.asarray(a, dtype=np.float32)
    e = np.exp(a - a.max(), dtype=np.float32)
    return (e / e.sum(dtype=np.float32)).astype(np.float32)


def make_in_maps(x, alpha_activ, alpha_weight, conv_weight, selected_channels):
    x = np.ascontiguousarray(np.asarray(x, dtype=np.float32).reshape(B, C, HW))
    ch = [int(v) for v in np.asarray(selected_channels).ravel()]
    chset = set(ch)
    nonsel = [c for c in range(C) if c not in chset]
    P = np.array(nonsel + ch, dtype=np.int64)  # sel channels at rows 248:256

    sel = x[:, ch, :]  # [32, 8, 4096] fp32 exact
    smn, smx = sel.min(), sel.max()
    wmat = np.asarray(conv_weight, dtype=np.float32).reshape(C, C)
    wmn, wmx = wmat.min(), wmat.max()

    cbrow = np.zeros((1, 16), dtype=np.float32)
    _host_consts(cbrow[0, 0:8], smn, smx, _softmax32(alpha_activ))
    _host_consts(cbrow[0, 8:16], wmn, wmx, _softmax32(alpha_weight))

    # W^T with permuted input channels, m-major chunks: [K0m0|K1m0|K0m1|K1m1]
    wperm = np.ascontiguousarray(wmat[:, P].T)  # [256(k), 256(m)]
    wt = np.ascontiguousarray(
        np.hstack([
            wperm[0:128, 0:128], wperm[128:256, 0:128],
            wperm[0:128, 128:256], wperm[128:256, 128:256],
        ])
    )

    xp = x[:, P, :].astype(np.float16)  # [32, 256, 4096] fp16, permuted

    in_maps = []
    for c in range(NCORES):
        xs = xp[c * BPC : (c + 1) * BPC]  # [4, 256, 4096]
        x0 = np.ascontiguousarray(xs[:, 0:128, :].transpose(1, 0, 2).reshape(128, -1))
        x1 = np.ascontiguousarray(xs[:, 128:256, :].transpose(1, 0, 2).reshape(128, -1))
        # selloc layout (q-outer): partition p = q*32 + b*8 + j holds
        # sel[core*4+b, j, q*1024 : (q+1)*1024]
        sl = sel[c * BPC : (c + 1) * BPC].reshape(BPC, NSEL, 4, 1024)
        selloc = np.ascontiguousarray(sl.transpose(2, 0, 1, 3).reshape(128, 1024))
        in_maps.append({"x0": x0, "x1": x1, "selloc": selloc, "wt": wt, "cb": cbrow})
    return ch, in_maps


def kernel(x, alpha_activ, alpha_weight, conv_weight, selected_channels):
    from concourse.bass_utils import run_bass_kernel_spmd

    ch, in_maps = make_in_maps(
        x, alpha_activ, alpha_weight, conv_weight, selected_channels
    )
    nc = build_program(ch)
    res = run_bass_kernel_spmd(nc, in_maps, core_ids=list(range(NCORES)))
    outs = [
        res.results[c]["out"].astype(np.float32).reshape(BPC, C, H, W)
        for c in range(NCORES)
    ]
    return np.concatenate(outs, axis=0)


# revision 77
# speedup vs baseline: 1.0169x; 1.0003x over previous
"""Trainium2 Bass kernel for MixActivConv2d (mixed-precision fake-quant + 1x1 conv).

Reference computation:
  sel = x[:, ch]                                   # gather 8 channels
  activ = sum_i softmax(aa)[i] * uq(sel, bit_i)    # global-minmax fake quant
  x_q = x with sel channels replaced by activ
  w_q = sum_i softmax(aw)[i] * uq(w, bit_i)
  out = conv1x1(x_q, w_q)  ==  w_q[256,256] @ x_q[b, 256, 4096]

Strategy (8 cores, data-parallel over batch, 4 batches/core):
  - channels permuted so the 8 selected channels sit in the last 8 rows of
    the second K-half; x streams in fp16, the GEMM runs in fp16 (1 cyc/row
    on the PE vs 4 for fp32), output is written fp16 and upcast on host
  - the fake-quant of the selected channels runs on device in exact fp32
    (per-op IEEE arithmetic, magic-number RNE rounding identical to the
    reference), producing fp16 activ rows that are DMA-scattered into the
    rhs stream before the K1 matmul passes
  - weight fake-quant also on device (fp32 exact -> fp16 lhsT)
  - global min/max of sel and W plus the handful of scalar constants
    (1/scale_i, folded softmax-blend factors) are computed host-side in
    exact IEEE fp32 (bit-identical to the on-device scalar chain they
    replace), so no cross-core reduction or collective is needed
  - DMA cost scales with bytes-per-partition, so transfers are shaped wide
    (128 partitions) and spread over the SP / ACT / Pool-SWDGE queues; ACT-
    and Pool-queue transfers occupy their engines, so the placement trades
    off against quant work and PSUM evictions (DVE/ACT only: GPSIMD cannot
    read PSUM on real TRN2)
"""

import sys
from contextlib import ExitStack

import numpy as np

sys.path.insert(0, "/opt/trn_rl_repo")

import concourse.bass as bass  # noqa: E402
import concourse.mybir as mybir  # noqa: E402
import concourse.tile as tile  # noqa: E402
from concourse import bacc  # noqa: E402

NCORES = 8
B, C, H, W = 32, 256, 64, 64
HW = H * W  # 4096
BPC = B // NCORES  # batches per core = 4
NSEL = 8
QMAX = (3.0, 15.0, 255.0)  # 2^bit - 1 for bits (2, 4, 8)
MAGIC = 12582912.0  # 1.5 * 2**23: x + MAGIC - MAGIC == rne-round(x) for |x| < 2^22
F32 = mybir.dt.float32
F16 = mybir.dt.float16
ALU = mybir.AluOpType
ACTF = mybir.ActivationFunctionType

# cb column layout per path (sel path at col 0, W path at col 8):
#  +0: -mn   +1..3: inv_i = 1/scale_i   +4..6: k_i = sw_i*scale_i   +7: mn


def _emit_quant(nc, tmps, src, cb, sc0, dst, dc0, cbase, ncols, danger, mid, tail):
    """Fake-quant src[:, sc0:sc0+ncols] -> dst[:, dc0:dc0+ncols] (fp16 out).

    Danger ops (pre-round, bit-exact IEEE per op, no fused nonzero
    scale+bias): u = src + (-mn); r_i = u*inv_i; rho_i = r_i + MAGIC.
    Run on DVE/gpsimd as tensor ops or on ACT as single-op activations
    (a neutral second operand keeps each op exactly IEEE).
    Mid ops (post-round; the MAGIC offset must come off BEFORE any scaling
    — exact there, catastrophic after): p_i = (rho_i - MAGIC)*k_i — fused
    tensor_scalar on DVE/gp, or two exact single-op activations on ACT.
    Tail (DVE/gp): s = p0 + p1;  dst = (p2 + mn) + s  (STT writes fp16).
    """
    u, rho = tmps
    ssl = (slice(0, 128), slice(sc0, sc0 + ncols))
    tsl = (slice(0, 128), slice(sc0, sc0 + ncols))
    dsl = (slice(0, 128), slice(dc0, dc0 + ncols))
    first = None
    if danger == "act":
        first = nc.scalar.activation(
            u[tsl], src[ssl], ACTF.Identity, bias=cb[:, cbase : cbase + 1]
        )
        for i in range(3):
            nc.scalar.activation(
                rho[i][tsl], u[tsl], ACTF.Copy,
                scale=cb[:, cbase + 1 + i : cbase + 2 + i],
            )
            nc.scalar.activation(rho[i][tsl], rho[i][tsl], ACTF.Copy, bias=MAGIC)
    else:
        eng = nc.vector if danger == "dve" else nc.gpsimd
        first = eng.tensor_scalar(
            u[tsl], src[ssl], cb[:, cbase : cbase + 1], None, op0=ALU.add
        )
        for i in range(3):
            eng.tensor_scalar(
                rho[i][tsl], u[tsl], cb[:, cbase + 1 + i : cbase + 2 + i], None,
                op0=ALU.mult,
            )
            eng.tensor_scalar(rho[i][tsl], rho[i][tsl], MAGIC, None, op0=ALU.add)
    if mid == "act":
        for i in range(3):
            nc.scalar.activation(rho[i][tsl], rho[i][tsl], ACTF.Copy, bias=-MAGIC)
            nc.scalar.activation(
                rho[i][tsl], rho[i][tsl], ACTF.Copy,
                scale=cb[:, cbase + 4 + i : cbase + 5 + i],
            )
    else:
        eng2 = nc.vector if mid == "dve" else nc.gpsimd
        for i in range(3):
            eng2.tensor_scalar(
                rho[i][tsl], rho[i][tsl], MAGIC, cb[:, cbase + 4 + i : cbase + 5 + i],
                op0=ALU.subtract, op1=ALU.mult,
            )
    if tail == "gp":
        # STT has no Pool opcode: use TT,TT,TS so the chunk never touches DVE
        nc.gpsimd.tensor_add(rho[0][tsl], rho[0][tsl], rho[1][tsl])
        nc.gpsimd.tensor_add(rho[2][tsl], rho[2][tsl], rho[0][tsl])
        last = nc.gpsimd.tensor_scalar(
            dst[dsl], rho[2][tsl], cb[:, cbase + 7 : cbase + 8], None, op0=ALU.add
        )
    else:
        nc.vector.tensor_add(rho[0][tsl], rho[0][tsl], rho[1][tsl])
        last = nc.vector.scalar_tensor_tensor(
            dst[dsl], rho[2][tsl], cb[:, cbase + 7 : cbase + 8], rho[0][tsl],
            op0=ALU.add, op1=ALU.add,
        )
    return first, last


def _kernel_body(ctx, tc, x0_ap, x1_ap, selloc_ap, w_ap, cb_ap, out_ap):
    nc = tc.nc

    const = ctx.enter_context(tc.tile_pool(name="const", bufs=1))
    stage = ctx.enter_context(tc.tile_pool(name="stage", bufs=4))
    psB = ctx.enter_context(tc.tile_pool(name="psB", bufs=4, space="PSUM"))

    # ---- input DMAs ----
    # SP queue: constants first (they gate all quant work), then the x
    # stream for b0/b1/b3 and b2's K1 half, with b1/b3 emitted after the
    # quant so each batch's scatter can sit right behind its x chunks.
    cb = const.tile([128, 16], F32)
    nc.sync.dma_start(cb[:], cb_ap)
    selloc = const.tile([128, 1024], F32)
    nc.sync.dma_start(selloc[:, 0:512], selloc_ap[:, 0:512])
    wtside = const.tile([128, 2 * C], F32)
    nc.sync.dma_start(wtside[:], w_ap)
    nc.scalar.dma_start(selloc[:, 512:1024], selloc_ap[:, 512:1024])

    rhs0 = const.tile([128, BPC * HW], F16, name="rhs0", tag="rhs0")
    rhs1 = const.tile([128, BPC * HW], F16, name="rhs1", tag="rhs1")

    def load_x(b, eng):
        cs = slice(b * HW, (b + 1) * HW)
        eng.dma_start(rhs0[:, cs], x0_ap[:, cs])
        eng.dma_start(rhs1[:, cs], x1_ap[:, cs])

    def scatter(b):
        # q-outer packed layout: activ rows q*32+b*8+j -> rhs1 row 120+j,
        # cols b*HW + q*1024.  One [8,1024] DMA per (b, q): per-partition
        # bytes stay small, which is what the DMA cost scales with.  Two
        # chunks each on the ACT and Pool queues run in parallel.
        # Must follow b's x1 chunk (WAW on the sel rows).
        for q in range(4):
            eng = (nc.scalar if q < 2 else nc.gpsimd) if b == 0 else nc.gpsimd
            eng.dma_start(
                rhs1[120:128, b * HW + q * 1024 : b * HW + (q + 1) * 1024],
                activ16[q * 32 + b * 8 : q * 32 + (b + 1) * 8, :],
            )

    load_x(0, nc.sync)

    # quant scratch (shared across chunks; chunks touch disjoint columns)
    tmps_s = (
        const.tile([128, 1024], F32, name="qus", tag="qus"),
        [const.tile([128, 1024], F32, name=f"qrs{i}", tag=f"qrs{i}") for i in range(3)],
    )
    tmps_w = (
        const.tile([128, 512], F32, name="quw", tag="quw"),
        [const.tile([128, 512], F32, name=f"qrw{i}", tag=f"qrw{i}") for i in range(3)],
    )

    # lhsT layout (m-major, matching host wt): [K0m0 | K1m0 | K0m1 | K1m1]
    lhsT = const.tile([128, 512], F16, name="lhsT", tag="lhsT")
    activ16 = const.tile([128, 1024], F16, name="activ16", tag="activ16")
    # ---- sel quant (packed [128,1024]: p = q*32 + b*8 + j, cols = hw%1024)
    # activ gates every K1 pass; W-m0 only gates the first K0 pass, which
    # the PE reaches later — so the Pool engine runs its sel share FIRST.
    _emit_quant(nc, tmps_s, selloc, cb, 0, activ16, 0, 0, 576, "dve", "dve", "dve")
    _emit_quant(nc, tmps_s, selloc, cb, 576, activ16, 576, 0, 176, "act", "act", "dve")
    selp_first, selp_last = _emit_quant(
        nc, tmps_s, selloc, cb, 752, activ16, 752, 0, 272, "gp", "gp", "gp"
    )

    wm0_first, _ = _emit_quant(
        nc, tmps_w, wtside, cb, 0, lhsT, 0, 8, 128, "gp", "gp", "gp"
    )
    _emit_quant(nc, tmps_w, wtside, cb, 128, lhsT, 128, 8, 128, "gp", "gp", "gp")
    from concourse.tile import add_dep_helper

    add_dep_helper(wm0_first.ins, selp_last.ins, sync=False, reason="pool: sel first")

    # W m1 chunk on DVE after its sel share (PE needs lhsT-m1 several us in)
    _emit_quant(nc, tmps_w, wtside, cb, 256, lhsT, 256, 8, 256, "dve", "dve", "dve")

    # x stream: b1, b3 and b2's K1 half on SP; b2's K0 half on the Pool
    # queue in its idle window between quant and evictions
    nc.gpsimd.dma_start(rhs0[:, 2 * HW : 3 * HW], x0_ap[:, 2 * HW : 3 * HW])
    with tc.high_priority():
        scatter(0)
    load_x(1, nc.sync)
    scatter(1)
    nc.sync.dma_start(rhs1[:, 2 * HW : 3 * HW], x1_ap[:, 2 * HW : 3 * HW])
    scatter(2)
    load_x(3, nc.sync)
    scatter(3)

    # ---- main GEMM: per (b, m): 4 psum tiles [128,1024], K0+K1, evict ----
    # NOTE: GPSIMD cannot read PSUM on real TRN2 — evictions are DVE/ACT only
    evict_sched = ["dve", "act"] * 16
    # out halves [128,2048] alternate queues so transfers overlap
    out_qs = {
        0: (nc.scalar, nc.gpsimd), 1: (nc.sync, nc.gpsimd),
        2: (nc.sync, nc.gpsimd), 3: (nc.sync, nc.scalar),
    }
    ei = 0
    for b in (0, 1, 2, 3):
        for m in range(2):
            outsb = stage.tile([128, HW], F16, name="outsb", tag="outsb")
            is_last = b == 3 and m == 1
            if is_last:
                # drain each tile right after its eviction; the final tile
                # evicts per half on DVE+ACT in parallel with quarter drains
                # so the post-PE tail is as short as possible
                for t in range(4):
                    pt = psB.tile([128, 1024], F32, name="ptile", tag="ptile")
                    for h in range(2):
                        c0 = b * HW + t * 1024 + h * 512
                        nc.tensor.matmul(
                            pt[:, h * 512 : (h + 1) * 512],
                            lhsT[:, m * 256 : m * 256 + 128],
                            rhs0[:, c0 : c0 + 512], start=True, stop=False,
                        )
                    for h in range(2):
                        c0 = b * HW + t * 1024 + h * 512
                        nc.tensor.matmul(
                            pt[:, h * 512 : (h + 1) * 512],
                            lhsT[:, m * 256 + 128 : m * 256 + 256],
                            rhs1[:, c0 : c0 + 512], start=False, stop=True,
                        )
                    if t == 3:
                        for h in range(2):
                            c1 = t * 1024 + h * 512
                            osl = outsb[:, c1 : c1 + 512]
                            if h == 0:
                                nc.scalar.copy(osl, pt[:, 0:512])
                            else:
                                nc.vector.tensor_copy(osl, pt[:, 512:1024])
                            eng = (nc.sync, nc.scalar)[h]
                            eng.dma_start(
                                out_ap[b, m * 128 : (m + 1) * 128, c1 : c1 + 512],
                                osl,
                            )
                        continue
                    osl = outsb[:, t * 1024 : (t + 1) * 1024]
                    evl = ("dve", "act", "dve")[t]
                    if evl == "act":
                        nc.scalar.copy(osl, pt[:])
                    else:
                        nc.vector.tensor_copy(osl, pt[:])
                    eng = (nc.sync, nc.scalar, nc.sync)[t]
                    eng.dma_start(
                        out_ap[b, m * 128 : (m + 1) * 128, t * 1024 : (t + 1) * 1024],
                        osl,
                    )
                continue
            for t in range(4):
                pt = psB.tile([128, 1024], F32, name="ptile", tag="ptile")
                for h in range(2):
                    c0 = b * HW + t * 1024 + h * 512
                    nc.tensor.matmul(
                        pt[:, h * 512 : (h + 1) * 512],
                        lhsT[:, m * 256 : m * 256 + 128],
                        rhs0[:, c0 : c0 + 512],
                        start=True, stop=False,
                    )
                for h in range(2):
                    c0 = b * HW + t * 1024 + h * 512
                    nc.tensor.matmul(
                        pt[:, h * 512 : (h + 1) * 512],
                        lhsT[:, m * 256 + 128 : m * 256 + 256],
                        rhs1[:, c0 : c0 + 512],
                        start=False, stop=True,
                    )
                ev = evict_sched[ei]
                ei += 1
                osl = outsb[:, t * 1024 : (t + 1) * 1024]
                if ev == "act":
                    nc.scalar.copy(osl, pt[:])
                elif ev == "dve":
                    nc.vector.tensor_copy(osl, pt[:])
                else:
                    nc.gpsimd.tensor_copy(osl, pt[:])
            for hh in range(2):
                out_qs[b][hh].dma_start(
                    out_ap[b, m * 128 : (m + 1) * 128, hh * 2048 : (hh + 1) * 2048],
                    outsb[:, hh * 2048 : (hh + 1) * 2048],
                )


def build_program(ch=None, reps=1):
    nc = bacc.Bacc(
        "TRN2", target_bir_lowering=False, debug=False, num_devices=NCORES
    )
    x0_t = nc.dram_tensor("x0", [128, BPC * HW], F16, kind="ExternalInput").ap()
    x1_t = nc.dram_tensor("x1", [128, BPC * HW], F16, kind="ExternalInput").ap()
    selloc_t = nc.dram_tensor("selloc", [128, 1024], F32, kind="ExternalInput").ap()
    w_t = nc.dram_tensor("wt", [128, 2 * C], F32, kind="ExternalInput").ap()
    cb_t = nc.dram_tensor("cb", [128, 16], F32, kind="ExternalInput").ap()
    out_t = nc.dram_tensor("out", [BPC, C, HW], F16, kind="ExternalOutput").ap()
    with tile.TileContext(nc) as tc:
        with ExitStack() as ctx:
            _kernel_body(ctx, tc, x0_t, x1_t, selloc_t, w_t, cb_t, out_t)
    nc.compile()
    return nc


def _f32(v):
    return np.float32(v)


def _host_consts(vals, mn, mx, sw):
    """Exact-IEEE fp32 constants for one quant path -> 8 floats."""
    rng = _f32(mx) - _f32(mn)
    scale = [rng / _f32(q) for q in QMAX]
    inv = [_f32(1.0) / s for s in scale]
    k = [_f32(sw[i]) * scale[i] for i in range(3)]
    vals[0] = -_f32(mn)
    vals[1:4] = inv
    vals[4:7] = k
    vals[7] = _f32(mn)


def _softmax32(a):
    a = np.asarray(a, dtype=np.float32)
    e = np.exp(a - a.max(), dtype=np.float32)
    return (e / e.sum(dtype=np.float32)).astype(np.float32)


def make_in_maps(x, alpha_activ, alpha_weight, conv_weight, selected_channels):
    x = np.ascontiguousarray(np.asarray(x, dtype=np.float32).reshape(B, C, HW))
    ch = [int(v) for v in np.asarray(selected_channels).ravel()]
    chset = set(ch)
    nonsel = [c for c in range(C) if c not in chset]
    P = np.array(nonsel + ch, dtype=np.int64)  # sel channels at rows 248:256

    sel = x[:, ch, :]  # [32, 8, 4096] fp32 exact
    smn, smx = sel.min(), sel.max()
    wmat = np.asarray(conv_weight, dtype=np.float32).reshape(C, C)
    wmn, wmx = wmat.min(), wmat.max()

    cbrow = np.zeros((1, 16), dtype=np.float32)
    _host_consts(cbrow[0, 0:8], smn, smx, _softmax32(alpha_activ))
    _host_consts(cbrow[0, 8:16], wmn, wmx, _softmax32(alpha_weight))
    cbfull = np.ascontiguousarray(np.tile(cbrow, (128, 1)))

    # W^T with permuted input channels, m-major chunks: [K0m0|K1m0|K0m1|K1m1]
    wperm = np.ascontiguousarray(wmat[:, P].T)  # [256(k), 256(m)]
    wt = np.ascontiguousarray(
        np.hstack([
            wperm[0:128, 0:128], wperm[128:256, 0:128],
            wperm[0:128, 128:256], wperm[128:256, 128:256],
        ])
    )

    xp = x[:, P, :].astype(np.float16)  # [32, 256, 4096] fp16, permuted

    in_maps = []
    for c in range(NCORES):
        xs = xp[c * BPC : (c + 1) * BPC]  # [4, 256, 4096]
        x0 = np.ascontiguousarray(xs[:, 0:128, :].transpose(1, 0, 2).reshape(128, -1))
        x1 = np.ascontiguousarray(xs[:, 128:256, :].transpose(1, 0, 2).reshape(128, -1))
        # selloc layout (q-outer): partition p = q*32 + b*8 + j holds
        # sel[core*4+b, j, q*1024 : (q+1)*1024]
        sl = sel[c * BPC : (c + 1) * BPC].reshape(BPC, NSEL, 4, 1024)
        selloc = np.ascontiguousarray(sl.transpose(2, 0, 1, 3).reshape(128, 1024))
        in_maps.append({"x0": x0, "x1": x1, "selloc": selloc, "wt": wt, "cb": cbfull})
    return ch, in_maps


def kernel(x, alpha_activ, alpha_weight, conv_weight, selected_channels):
    from concourse.bass_utils import run_bass_kernel_spmd

    ch, in_maps = make_in_maps(
        x, alpha_activ, alpha_weight, conv_weight, selected_channels
    )
    nc = build_program(ch)
    res = run_bass_kernel_spmd(nc, in_maps, core_ids=list(range(NCORES)))
    outs = [
        res.results[c]["out"].astype(np.float32).reshape(BPC, C, H, W)
        for c in range(NCORES)
    ]
    return np.concatenate(outs, axis=0)
